# revision 28
# baseline (speedup 1.0000x reference)
"""Trainium2 Bass kernel for DiffusionMSAE (top-k masking autoencoder).

Computes, for x [32,1280,16,16]:
  z = x_flat @ W_enc + b_enc ; r = relu(z)
  feats_k = top-k masking of r (k = 16, 32)
  recons_k = feats_k @ W_dec + b_dec
Returns (feats [2,8192,5120] f32, recons [2,32,1280,16,16] f32).

Strategy: data-parallel over tokens across 8 NeuronCores (1024 tokens/core,
4 batch images/core), weights replicated. Per core, tokens are processed in
2 groups of 512 (4 tiles of 128):
  - Encoder: PE matmul with z[tok, h] layout (x^T is the natural memory
    layout of x). Run as ONE 30-chunk bf16 PSUM accumulation implementing
    the exact split x_b@W_b + x_b@W_lo + x_lo@W_b (host-prepared K-concat),
    which reproduces fp32 z to ~5e-6 — needed so top-k selection agrees
    with the fp32 reference near thresholds.
  - Top-k thresholds on DVE: per-128-chunk max8 -> 320 candidates/row ->
    4 rounds of max8+match_replace -> 16th/32nd largest. Exact whenever no
    128-chunk holds >8 of a row's top-32 (holds with margin for this data;
    relu ties at 0 are harmless because masking keeps values, not indices).
  - Masking: tensor_scalar(is_ge, per-partition threshold) * r, in place.
    k=32 first; k=16 then masks feats32 in place (t16 >= t32 > 0).
  - Decoder: masked values cast to fp16, transposed to [h, tok] chunks via
    xbar DMA-transpose, PE matmul against host-prepacked chunk-major fp16
    W_dec producing recons in [c, tok] layout = the natural recons layout.
"""

import sys

sys.path.insert(0, "/opt/trn_rl_repo")

import numpy as np
import ml_dtypes

N_CORES = 8
C_IN = 1280
HID = 5120
B = 32
HW = 256  # 16*16
N_TOK = B * HW  # 8192
TPC = N_TOK // N_CORES  # tokens per core: 1024
G = 512  # token group
NT = G // 128  # tiles per group: 4
NG = TPC // G  # groups per core: 2
HS = 512  # encoder h-slice width
NS = HID // HS  # 20
NCH = HID // 128  # 40 h-chunks
K_VALUES = (16, 32)

_BUILT = {}
LAST_EXEC_NS = None
LAST_RESULTS = None


def _build(kc_tot, with_benc, with_bdec, timing=False):
    """Build + compile the per-core Bass graph. kc_tot = K chunks (30 for
    split encoder; +1 if bias chunk). timing=True replaces the big inputs
    with Internal DRAM (garbage values, same instruction stream) so device
    time can be measured without per-exec input shipping."""
    import concourse.bacc as bacc
    import concourse.mybir as mybir
    from concourse.tile import TileContext

    F32 = mybir.dt.float32
    F16 = mybir.dt.float16
    BF16 = mybir.dt.bfloat16

    nc = bacc.Bacc("TRN2", target_bir_lowering=False, debug=False, num_devices=N_CORES)

    kind_big = "Internal" if timing else "ExternalInput"
    xcat = nc.dram_tensor("xcat", [kc_tot * 128, TPC], BF16, kind=kind_big)
    wcat = nc.dram_tensor("wcat", [kc_tot * 128, HID], BF16, kind=kind_big)
    wd = nc.dram_tensor("wd", [NCH, 128, C_IN], F16, kind=kind_big)
    dummy = None
    if timing:
        dummy = nc.dram_tensor("tdummy", [128, 16], F32, kind="ExternalInput")
    bdec = None
    if with_bdec:
        bdec = nc.dram_tensor("bdec", [10, 128], F32, kind="ExternalInput")
    feats = nc.dram_tensor("feats", [2, TPC, HID], F32, kind="ExternalOutput")
    recons = nc.dram_tensor("recons", [2, TPC // HW, C_IN, HW], F32, kind="ExternalOutput")

    def self_copy_recons(nc, mybir, spool, dp, bdec_sb, with_bdec, cc, recons, ki, g, G, HW):
        ro = spool.tile([128, G], F32, tag="ro", name=f"ro{cc}_{g}_{ki}")
        if with_bdec:
            nc.vector.tensor_scalar(
                ro[:], dp[:], bdec_sb[:, cc : cc + 1], None, op0=mybir.AluOpType.add
            )
        else:
            nc.scalar.copy(ro[:], dp[:])
        nc.sync.dma_start(
            recons.ap()[
                ki, g * (G // HW) : (g + 1) * (G // HW), cc * 128 : (cc + 1) * 128, :
            ].rearrange("b c hw -> c b hw"),
            ro[:].rearrange("c (b hw) -> c b hw", hw=HW),
        )

    with TileContext(nc) as tc:
        with tc.tile_pool(name="perm", bufs=1) as perm, tc.tile_pool(
            name="small", bufs=4
        ) as small, tc.tile_pool(name="ps", bufs=8, space="PSUM") as psp:
            # persistent: r tiles (one per tile slot in the current group)
            r = [perm.tile([128, HID], F32, tag=f"r{t}", name=f"r{t}") for t in range(NT)]
            r32 = [perm.tile([128, 32], F32, tag=f"r32_{t}", name=f"r32_{t}") for t in range(NT)]
            cand = [
                perm.tile([128, NCH * 8], F32, tag=f"cand{t}", name=f"cand{t}")
                for t in range(NT)
            ]
            if timing:
                dtile = perm.tile([128, 16], F32, tag="dummy")
                nc.sync.dma_start(dtile[:], dummy.ap())
                nc.sync.dma_start(feats.ap()[0, 0:128, 0:16], dtile[:])
            bdec_sb = None
            if with_bdec:
                bdec_sb = perm.tile([128, 10], F32, tag="bdec")
                nc.sync.dma_start(
                    bdec_sb[:], bdec.ap().rearrange("cc p -> p cc")
                )

            for g in range(NG):
                # ---------------- encode group g -> r[t] ----------------
                with tc.tile_pool(name=f"enc{g}", bufs=1) as ep, tc.tile_pool(
                    name=f"encw{g}", bufs=2
                ) as ewp:
                    xg = ep.tile([128, kc_tot, G], BF16, tag="xg")
                    for t in range(NT):
                        nc.gpsimd.dma_start(
                            xg[:, :, t * 128 : (t + 1) * 128],
                            xcat.ap()[
                                :, g * G + t * 128 : g * G + (t + 1) * 128
                            ].rearrange("(c p) m -> p c m", p=128),
                        )
                    for s in range(NS):
                        ws = ewp.tile([128, kc_tot, HS], BF16, tag="ws")
                        wsl = wcat.ap()[:, s * HS : (s + 1) * HS].rearrange(
                            "(c p) n -> p c n", p=128
                        )
                        nc.sync.dma_start(ws[:, :5, :], wsl[:, :5, :])
                        nc.sync.dma_start(ws[:, 5:, :], wsl[:, 5:, :])
                        for t in range(NT):
                            zp = psp.tile([128, HS], F32, tag="ps", name=f"zp_{g}_{s}_{t}")
                            for c in range(kc_tot):
                                nc.tensor.matmul(
                                    zp[:],
                                    xg[:, c, t * 128 : (t + 1) * 128],
                                    ws[:, c, :],
                                    start=(c == 0),
                                    stop=(c == kc_tot - 1),
                                )
                            nc.scalar.activation(
                                r[t][:, s * HS : (s + 1) * HS],
                                zp[:],
                                mybir.ActivationFunctionType.Relu,
                            )
                            # candidate extraction overlaps the encoder
                            ch0 = s * HS // 128
                            for ch in range(ch0, ch0 + HS // 128):
                                nc.vector.max(
                                    out=cand[t][:, ch * 8 : (ch + 1) * 8],
                                    in_=r[t][:, ch * 128 : (ch + 1) * 128],
                                )

                # ---------------- top-k thresholds per tile ----------------
                for t in range(NT):
                    work = small.tile([128, NCH * 8], F32, tag="work", bufs=1)
                    nc.vector.tensor_copy(work[:], cand[t][:])
                    for i in range(4):
                        nc.vector.max(out=r32[t][:, i * 8 : (i + 1) * 8], in_=work[:])
                        if i < 3:
                            nc.vector.match_replace(
                                out=work[:],
                                in_to_replace=r32[t][:, i * 8 : (i + 1) * 8],
                                in_values=work[:],
                                imm_value=-1.0,
                            )

                # ------------- mask + feats out + decode per k -------------
                with tc.tile_pool(name=f"dec{g}", bufs=1) as dpool, tc.tile_pool(
                    name=f"decs{g}", bufs=3
                ) as spool, tc.tile_pool(name=f"decw{g}", bufs=3) as wdp:
                    for ki, col in ((1, 31), (0, 15)):
                        # masks + feats store + fp16 transpose, q-block major so
                        # decode (h-outer) can start on early chunks
                        ftq = [
                            dpool.tile([128, 10, G], F16, tag=f"ftq{q}", name=f"ftq{q}_{g}_{ki}")
                            for q in range(4)
                        ]
                        for q in range(4):
                            sl = slice(q * 1280, (q + 1) * 1280)
                            for t in range(NT):
                                thr = r32[t][:, col : col + 1]
                                m = spool.tile([128, 1280], F32, tag="m", name=f"m_{g}_{ki}_{q}_{t}")
                                nc.vector.tensor_scalar(
                                    m[:],
                                    r[t][:, sl],
                                    thr,
                                    None,
                                    op0=mybir.AluOpType.is_ge,
                                )
                                nc.vector.tensor_mul(r[t][:, sl], m[:], r[t][:, sl])
                                nc.sync.dma_start(
                                    feats.ap()[
                                        ki,
                                        g * G + t * 128 : g * G + (t + 1) * 128,
                                        sl,
                                    ],
                                    r[t][:, sl],
                                )
                                h16 = spool.tile([128, 1280], F16, tag="h16", name=f"h16_{g}_{ki}_{q}_{t}")
                                nc.scalar.copy(h16[:], r[t][:, sl])
                                nc.scalar.dma_start_transpose(
                                    ftq[q][:, :, t * 128 : (t + 1) * 128],
                                    h16[:],
                                )
                        # decode, h-outer: sweep A covers c-chunks 0..7 in 8 psum
                        # banks; sweep B covers c-chunks 8,9.
                        dpA = [
                            psp.tile([128, G], F32, tag="ps", name=f"dpA{cc}_{g}_{ki}")
                            for cc in range(8)
                        ]
                        for h0 in range(0, NCH, 4):
                            wdt = wdp.tile([128, 4, C_IN], F16, tag="wdt")
                            nc.gpsimd.dma_start(
                                wdt[:],
                                wd.ap()[h0 : h0 + 4].rearrange("hh p c -> p hh c"),
                            )
                            for hh in range(4):
                                h = h0 + hh
                                for cc in range(8):
                                    nc.tensor.matmul(
                                        dpA[cc][:],
                                        wdt[:, hh, cc * 128 : (cc + 1) * 128],
                                        ftq[h // 10][:, h % 10, :],
                                        start=(h == 0),
                                        stop=(h == NCH - 1),
                                    )
                        for cc in range(8):
                            self_copy_recons(
                                nc, mybir, spool, dpA[cc], bdec_sb, with_bdec, cc,
                                recons, ki, g, G, HW,
                            )
                        dpB = [
                            psp.tile([128, G], F32, tag="ps", name=f"dpB{cc}_{g}_{ki}")
                            for cc in range(2)
                        ]
                        wdb = wdp.tile([128, NCH, 256], F16, tag="wdb", bufs=1)
                        nc.gpsimd.dma_start(
                            wdb[:],
                            wd.ap()[:, :, 1024:1280].rearrange("hh p c -> p hh c"),
                        )
                        for h in range(NCH):
                            for cc in range(2):
                                nc.tensor.matmul(
                                    dpB[cc][:],
                                    wdb[:, h, cc * 128 : (cc + 1) * 128],
                                    ftq[h // 10][:, h % 10, :],
                                    start=(h == 0),
                                    stop=(h == NCH - 1),
                                )
                        for cc in range(2):
                            self_copy_recons(
                                nc, mybir, spool, dpB[cc], bdec_sb, with_bdec, 8 + cc,
                                recons, ki, g, G, HW,
                            )

    nc.compile()
    return nc


def _host_prep(x, W_enc, b_enc, W_dec, b_dec):
    """Build per-core input maps. Encoder split: z = xb@Wb + xb@Wlo + xlo@Wb
    (+ optional bias chunk), all bf16, K-concatenated."""
    bf16 = ml_dtypes.bfloat16
    xT = np.ascontiguousarray(
        x.reshape(B, C_IN, HW).transpose(1, 0, 2).reshape(C_IN, N_TOK)
    )
    xb = xT.astype(bf16)
    xlo = (xT - xb.astype(np.float32)).astype(bf16)
    Wb = W_enc.astype(bf16)
    Wlo = (W_enc - Wb.astype(np.float32)).astype(bf16)

    with_benc = bool(np.any(b_enc != 0.0))
    with_bdec = bool(np.any(b_dec != 0.0))

    xcat_blocks = [xb, xb, xlo]
    wcat_blocks = [Wb, Wlo, Wb]
    if with_benc:
        ones_blk = np.zeros((128, N_TOK), dtype=bf16)
        ones_blk[0, :] = 1.0
        b_blk = np.zeros((128, HID), dtype=np.float32)
        b_blk[0, :] = b_enc
        xcat_blocks.append(ones_blk)
        wcat_blocks.append(b_blk.astype(bf16))
    xcat = np.concatenate(xcat_blocks, axis=0)
    wcat = np.concatenate(wcat_blocks, axis=0)
    kc_tot = xcat.shape[0] // 128

    wd_packed = np.ascontiguousarray(
        W_dec.astype(np.float16).reshape(NCH, 128, C_IN)
    )

    in_maps = []
    for i in range(N_CORES):
        m = {
            "xcat": np.ascontiguousarray(xcat[:, i * TPC : (i + 1) * TPC]),
            "wcat": wcat,
            "wd": wd_packed,
        }
        if with_bdec:
            m["bdec"] = np.ascontiguousarray(b_dec.reshape(10, 128))
        in_maps.append(m)
    return in_maps, kc_tot, with_benc, with_bdec


def kernel(x, W_enc, b_enc, W_dec, b_dec, _trace=False):
    global LAST_EXEC_NS, LAST_RESULTS
    from concourse.bass_utils import run_bass_kernel_spmd

    x = np.asarray(x, dtype=np.float32)
    W_enc = np.asarray(W_enc, dtype=np.float32)
    b_enc = np.asarray(b_enc, dtype=np.float32)
    W_dec = np.asarray(W_dec, dtype=np.float32)
    b_dec = np.asarray(b_dec, dtype=np.float32)

    in_maps, kc_tot, with_benc, with_bdec = _host_prep(x, W_enc, b_enc, W_dec, b_dec)
    key = (kc_tot, with_benc, with_bdec)
    if key not in _BUILT:
        _BUILT[key] = _build(kc_tot, with_benc, with_bdec)
    nc = _BUILT[key]

    res = run_bass_kernel_spmd(
        nc, in_maps, core_ids=list(range(N_CORES)), trace=_trace
    )
    LAST_EXEC_NS = res.exec_time_ns
    LAST_RESULTS = res

    feats = np.empty((2, N_TOK, HID), dtype=np.float32)
    recons = np.empty((2, B, C_IN, 16, 16), dtype=np.float32)
    bpc = TPC // HW  # batch images per core: 4
    for i, r in enumerate(res.results):
        feats[:, i * TPC : (i + 1) * TPC, :] = r["feats"]
        recons[:, i * bpc : (i + 1) * bpc] = r["recons"].reshape(
            2, bpc, C_IN, 16, 16
        )
    return feats, recons
